# revision 12
# baseline (speedup 1.0000x reference)
"""Trainium2 Bass kernel for nn_E4_C4 (C4-equivariant involution CNN).

Contract: kernel(**inputs) takes FULL unsharded inputs (as produced by
setup_inputs) and returns the FULL output [8, 512, 32, 32] fp32.

Strategy (data-parallel over batch, 1 batch element per core, 8 cores):
  per core, channels on partitions, spatial tap-shifts as free-dim offsets
  into zero-padded v planes:
    1. t  = W1 @ x           (PE GEMM fp32r, M=256 K=512 N=1024)
    2. GroupNorm+ReLU        (DVE bn_stats + tiny PE grouping matmuls +
                              ACT per-partition scale/bias apply -> t1 bf16)
    3. v  = Wv @ x           (PE GEMM fp32r) -> three padded 38x38 copies:
                              bf16 even/odd-parity pair (for DVE 2x-mode
                              reads) + fp32 (for Pool/1x paths)
    4. involution, r-PAIR interleaved (r=0/1 share c2 slot 0 on partition
       rows 0-63/64-127 -> consecutive wmap matmuls land on disjoint PE
       row-groups and overlap in the systolic array):
       per tap p and rotation r:
         wrep = c2rep[r,p] @ t1    (PE bf16, K=64, N=1024 -> PSUM fp32;
                                    channel replication + rot90 fused into
                                    host-prepared lhsT)
       then one of (balancing DVE/ACT/Pool):
         A: DVE multiply straight from PSUM (1x) -> prod bf16
         B: ACT evicts PSUM->SBUF bf16, DVE multiplies bf16 pairs (2x mode)
         D: like B but accumulated into an SBUF bf16 accumulator on DVE
         C: ACT evicts fp32, Pool (GPSIMD) multiply+accumulate in SBUF
       A/B prods accumulate into PSUM via PE identity matmuls (bf16 rhs);
       C/D accumulators fold in at the end via identity matmuls.
  Host side: C4-lift of the 1x1 weights, channel reorders, replication,
  rot90 permutations; final gather + channel re-order to reference layout.
"""

import os
from collections import deque
from contextlib import ExitStack

import numpy as np

import concourse.bacc as bacc
import concourse.bass as bass
import concourse.tile as tile
from concourse import mybir
from concourse.bass_utils import run_bass_kernel_spmd

# ---- problem constants (hardcoded per contract) ----
B = 8
CIN = 128
COUT = 128
KK = 7
R = 2
G = 8
GC = 16
H = W = 32
S = H * W  # 1024
EPS = 1e-5
NCORES = 8
F32 = mybir.dt.float32
F32R_G = mybir.dt.float32r
BF16 = mybir.dt.bfloat16

# Per tap-step p the two rotations of the pair are routed through DIFFERENT
# engine chains so the per-step consumers run in parallel:
#   A: DVE multiplies straight from PSUM (1x), B: ACT evicts then DVE 2x,
#   C: ACT evicts fp32 then Pool multiply+accumulate.
# The 7-step rotation balances DVE / ACT / Pool totals.
PATTERN = list(os.environ.get("KRN_PATTERN", "ABABCBA"))
PROD_BUFS = int(os.environ.get("KRN_PROD_BUFS", "8"))
WSB_BUFS = int(os.environ.get("KRN_WSB_BUFS", "3"))
ACC_LAG = int(os.environ.get("KRN_ACC_LAG", "6"))


# ------------------------------------------------------------------ host prep
def _c4_lift_np(w):
    Wr = np.stack([np.roll(w, r, axis=-1) for r in range(4)], axis=1)  # [o,4,i,4]
    o, _, i, _ = Wr.shape
    return Wr.reshape(o * 4, i * 4)


def _host_prep(v_w, c1_w, gn_g, gn_b, c2_w, c2_b):
    W1 = _c4_lift_np(np.asarray(c1_w, np.float32))  # [256, 512], rows c*4+r
    # rows c*4+r -> r-major (r*64+c)
    W1_r = W1.reshape(64, 4, 512).transpose(1, 0, 2).reshape(256, 512)
    W1T = np.ascontiguousarray(W1_r.T)  # [512, 256]

    Wv = _c4_lift_np(np.asarray(v_w, np.float32))  # [512, 512], rows (g*16+c)*4+r
    Wv_r = Wv.reshape(128, 4, 512).transpose(1, 0, 2).reshape(512, 512)
    WvT = np.ascontiguousarray(Wv_r.T)  # [512, 512]

    gam_r = np.ascontiguousarray(
        np.asarray(gn_g, np.float32).reshape(64, 4).T.reshape(2, 128).T
    )  # [128, 2]  col t holds channels t*128..t*128+127 in r-major order
    bet_r = np.ascontiguousarray(
        np.asarray(gn_b, np.float32).reshape(64, 4).T.reshape(2, 128).T
    )

    c2_w = np.asarray(c2_w, np.float32)
    c2_b = np.asarray(c2_b, np.float32)
    c2rep = np.zeros((128, 2, 49, 128), np.float32)
    bias_rep = np.zeros((128, 4, 49), np.float32)
    m_idx = np.arange(128)
    for r in range(4):
        perm = np.rot90(np.arange(49).reshape(7, 7), k=r).flatten()
        base = 64 * (r % 2)
        slot = r // 2
        for p in range(49):
            src_rows = (m_idx // 16) * 49 + perm[p]
            c2rep[base : base + 64, slot, p, :] = c2_w[src_rows, :].T
            bias_rep[:, r, p] = c2_b[src_rows]

    i128 = np.eye(128, dtype=np.float32)
    gmat = np.zeros((128, 64), np.float32)
    gmat[np.arange(128), np.arange(128) % 64] = 0.25
    emat = np.zeros((64, 128), np.float32)
    emat[np.arange(128) % 64, np.arange(128)] = 1.0
    return W1T, WvT, gam_r, bet_r, c2rep, bias_rep, i128, gmat, emat


def _tap_mode(p_idx):
    return PATTERN[p_idx % len(PATTERN)]


# ------------------------------------------------------------------ v2 build
def _build_module_v2(loop_n=1):
    nc = bacc.Bacc(None)

    x_d = nc.dram_tensor("x", [512, S], F32R_G, kind="ExternalInput")
    w1t_d = nc.dram_tensor("w1t", [512, 256], F32R_G, kind="ExternalInput")
    wvt_d = nc.dram_tensor("wvt", [512, 512], F32R_G, kind="ExternalInput")
    c2r_d = nc.dram_tensor("c2rep", [128, 2, 49, 128], BF16, kind="ExternalInput")
    gam_d = nc.dram_tensor("gam", [128, 2], F32, kind="ExternalInput")
    bet_d = nc.dram_tensor("bet", [128, 2], F32, kind="ExternalInput")
    i128_d = nc.dram_tensor("i128", [128, 128], BF16, kind="ExternalInput")
    i128f_d = nc.dram_tensor("i128f", [128, 128], F32R_G, kind="ExternalInput")
    gm_d = nc.dram_tensor("gmat", [128, 64], F32, kind="ExternalInput")
    em_d = nc.dram_tensor("emat", [64, 128], F32, kind="ExternalInput")
    out_d = nc.dram_tensor("out", [512, S], F32, kind="ExternalOutput")

    PAD = 38 * 38  # 1444

    with tile.TileContext(nc) as tc, ExitStack() as ctx:
        if loop_n > 1:
            ctx.enter_context(tc.For_i(0, loop_n, 1))
        consts = ctx.enter_context(tc.tile_pool(name="consts", bufs=1))
        sb = ctx.enter_context(tc.tile_pool(name="sb", bufs=1))
        small = ctx.enter_context(tc.tile_pool(name="small", bufs=8))
        pp = ctx.enter_context(tc.tile_pool(name="pp", bufs=PROD_BUFS))
        wsbB = ctx.enter_context(tc.tile_pool(name="wsbB", bufs=WSB_BUFS))
        wsbC = ctx.enter_context(tc.tile_pool(name="wsbC", bufs=2))
        accp = ctx.enter_context(tc.tile_pool(name="accp", bufs=1))
        outs = ctx.enter_context(tc.tile_pool(name="outs", bufs=2))
        phase1_psum = tc.tile_pool(name="psA", bufs=2, space="PSUM")
        psA = phase1_psum.__enter__()
        stat_psum = tc.tile_pool(name="psStat", bufs=1, space="PSUM")
        psStat = stat_psum.__enter__()

        AL = mybir.AluOpType

        # ---- load weights/constants into SBUF
        x_sb = sb.tile([128, 4, S], F32R_G)
        w1t_sb = sb.tile([128, 4, 256], F32R_G)
        wvt_sb = sb.tile([128, 4, 512], F32R_G)
        dma_engs = [nc.sync, nc.sync, nc.sync, nc.sync]
        for kt in range(4):
            dma_engs[kt].dma_start(
                out=x_sb[:, kt, :], in_=x_d[kt * 128 : (kt + 1) * 128, :]
            )
            dma_engs[(kt + 1) % 4].dma_start(
                out=w1t_sb[:, kt, :], in_=w1t_d[kt * 128 : (kt + 1) * 128, :]
            )
            dma_engs[(kt + 2) % 4].dma_start(
                out=wvt_sb[:, kt, :], in_=wvt_d[kt * 128 : (kt + 1) * 128, :]
            )
        c2r_sb = sb.tile([128, 2, 49, 128], BF16)
        # slot 0 (r=0/1) chunks first so pair-1 taps aren't gated on the rest
        for sl in range(2):
            for pc in range(4):
                ps0, ps1 = pc * 13, min((pc + 1) * 13, 49)
                dma_engs[(sl * 4 + pc) % 4].dma_start(
                    out=c2r_sb[:, sl, ps0:ps1, :], in_=c2r_d[:, sl, ps0:ps1, :]
                )
        gam_sb = consts.tile([128, 2], F32)
        nc.sync.dma_start(out=gam_sb, in_=gam_d[:])
        bet_sb = consts.tile([128, 2], F32)
        nc.sync.dma_start(out=bet_sb, in_=bet_d[:])
        i128_sb = consts.tile([128, 128], BF16)
        nc.sync.dma_start(out=i128_sb, in_=i128_d[:])
        i128f_sb = consts.tile([128, 128], F32R_G)
        nc.sync.dma_start(out=i128f_sb, in_=i128f_d[:])
        gm_sb = consts.tile([128, 64], F32)
        nc.sync.dma_start(out=gm_sb, in_=gm_d[:])
        em_sb = consts.tile([64, 128], F32)
        nc.sync.dma_start(out=em_sb, in_=em_d[:])

        eps_t = consts.tile([64, 1], F32)
        nc.vector.memset(eps_t, EPS)

        # warm the ACT function tables under the DMA shadow
        warm = consts.tile([1, 1], F32)
        nc.vector.memset(warm, 1.0)
        nc.scalar.activation(out=warm, in_=warm, func=mybir.ActivationFunctionType.Relu)
        nc.scalar.activation(out=warm, in_=warm, func=mybir.ActivationFunctionType.Sqrt)

        # padded v planes: bf16 even/odd parity pair + fp32 (pool / 1x paths)
        vpadE = sb.tile([128, 4, PAD], BF16)
        vpadO = sb.tile([128, 4, PAD], BF16)
        vpadP = sb.tile([128, 4, PAD], F32)
        nc.gpsimd.memset(vpadE.bitcast(mybir.dt.uint32), 0)
        nc.gpsimd.memset(vpadO.bitcast(mybir.dt.uint32), 0)
        nc.gpsimd.memset(vpadP, 0.0)

        # ---- GEMM1: t [256, 1024]; both M-tiles stay in PSUM through GN
        ps_t = []
        for mt in range(2):
            pt = psA.tile([128, S], F32, tag="mm_out")
            for nh in range(2):
                for kt in range(4):
                    nc.tensor.matmul(
                        pt[:, nh * 512 : (nh + 1) * 512],
                        lhsT=w1t_sb[:, kt, mt * 128 : (mt + 1) * 128],
                        rhs=x_sb[:, kt, nh * 512 : (nh + 1) * 512],
                        start=(kt == 0),
                        stop=(kt == 3),
                    )
            ps_t.append(pt)

        # ---- GroupNorm stats (read PSUM directly; m2 assembled in one STT)
        stats = []
        for t in range(2):
            st6 = small.tile([128, 2, 6], F32, tag="st6")
            for hh in range(2):
                nc.vector.bn_stats(
                    out=st6[:, hh, :], in_=ps_t[t][:, hh * 512 : (hh + 1) * 512]
                )
            mv = small.tile([128, 2], F32, tag="mv")
            nc.vector.bn_aggr(out=mv, in_=st6)
            # mv[:,1] <- mean^2 + var  (in-place; mv becomes [mean, m2])
            nc.vector.scalar_tensor_tensor(
                out=mv[:, 1:2],
                in0=mv[:, 0:1],
                scalar=mv[:, 0:1],
                in1=mv[:, 1:2],
                op0=AL.mult,
                op1=AL.add,
            )
            stats.append(mv)

        ps_g = psStat.tile([64, 2], F32, tag="gstat")
        for t in range(2):
            nc.tensor.matmul(
                ps_g, lhsT=gm_sb, rhs=stats[t], start=(t == 0), stop=(t == 1)
            )
        # group mean / m2 -> rstd
        gss = small.tile([64, 2], F32, tag="gss")
        nc.vector.tensor_copy(out=gss, in_=ps_g)  # evacuate PSUM
        gmv = small.tile([64, 2], F32, tag="gmv")  # [mean_g, rstd_g]
        nc.vector.tensor_copy(out=gmv[:, 0:1], in_=gss[:, 0:1])
        gv = small.tile([64, 1], F32, tag="gv")
        nc.vector.tensor_mul(out=gv, in0=gss[:, 0:1], in1=gss[:, 0:1])
        nc.vector.tensor_sub(out=gv, in0=gss[:, 1:2], in1=gv)
        nc.scalar.activation(
            out=gv, in_=gv, func=mybir.ActivationFunctionType.Sqrt, bias=eps_t, scale=1.0
        )
        nc.vector.reciprocal(out=gmv[:, 1:2], in_=gv)

        ps_e = psStat.tile([128, 2], F32, tag="gstat")
        nc.tensor.matmul(ps_e, lhsT=em_sb, rhs=gmv, start=True, stop=True)

        # per-partition scale/bias; apply GN + ReLU into t1
        t1_sb = sb.tile([128, 2, S], BF16)
        scb = small.tile([128, 2, 2], F32, tag="scb")
        for t in range(2):
            nc.vector.tensor_mul(
                out=scb[:, t, 0:1], in0=ps_e[:, 1:2], in1=gam_sb[:, t : t + 1]
            )
            nc.vector.tensor_mul(out=scb[:, t, 1:2], in0=ps_e[:, 0:1], in1=scb[:, t, 0:1])
            nc.vector.tensor_sub(
                out=scb[:, t, 1:2], in0=bet_sb[:, t : t + 1], in1=scb[:, t, 1:2]
            )
            nc.scalar.activation(
                out=t1_sb[:, t, :],
                in_=ps_t[t][:, :],
                func=mybir.ActivationFunctionType.Relu,
                scale=scb[:, t, 0:1],
                bias=scb[:, t, 1:2],
            )

        # ---- GEMMv r=0..3 -> padded v planes (bf16 E/O parity pair + fp32)
        for r in range(4):
            ps_v = psA.tile([128, S], F32, tag="mm_out")
            for nh in range(2):
                for kt in range(4):
                    nc.tensor.matmul(
                        ps_v[:, nh * 512 : (nh + 1) * 512],
                        lhsT=wvt_sb[:, kt, r * 128 : (r + 1) * 128],
                        rhs=x_sb[:, kt, nh * 512 : (nh + 1) * 512],
                        start=(kt == 0),
                        stop=(kt == 3),
                    )
            ps_v3 = ps_v.rearrange("q (y x) -> q y x", x=32)
            for vt, c0 in ((vpadE, 3), (vpadO, 4), (vpadP, 3)):
                vint = vt[:, r, :].rearrange("q (yy xx) -> q yy xx", xx=38)[
                    :, 3:35, c0 : c0 + 32
                ]
                nc.scalar.activation(
                    out=vint, in_=ps_v3, func=mybir.ActivationFunctionType.Copy
                )

        # phase-1 PSUM pools close here; the involution reuses their banks
        stat_psum.__exit__(None, None, None)
        phase1_psum.__exit__(None, None, None)
        psW = ctx.enter_context(tc.tile_pool(name="psW", bufs=3, space="PSUM"))
        psO = ctx.enter_context(tc.tile_pool(name="psO", bufs=1, space="PSUM"))

        vpadE4 = [vpadE[:, r, :].rearrange("q (yy xx) -> q yy xx", xx=38) for r in range(4)]
        vpadO4 = [vpadO[:, r, :].rearrange("q (yy xx) -> q yy xx", xx=38) for r in range(4)]
        vpadP4 = [vpadP[:, r, :].rearrange("q (yy xx) -> q yy xx", xx=38) for r in range(4)]

        # ---- involution: one rotation r at a time; psW ring-3 gives the
        # PE three taps of lookahead so the per-tap chain latency
        # (wmap matmul -> ACT/DVE consume -> PE accumulate) stays hidden
        for r in range(4):
            slot = r // 2
            kb = 64 * (r % 2)
            out_ps = psO.tile([128, S], F32, tag="o", name="out_ps")
            acc_started = [False, False]
            n_acc_mms = [0, 0]
            pool_acc = None
            modes = [_tap_mode(p) for p in range(49)]
            # accumulation groups are per PSUM bank: each spatial half of
            # out_ps needs its own start=True first matmul and stop=True last
            total_mms = sum(m in ("A", "B") for m in modes) + ("C" in modes)

            pending = deque()  # prod tiles awaiting PE accumulation

            def emit_acc(prod, lhsT=None):
                for nh in range(2):
                    n_acc_mms[nh] += 1
                    nc.tensor.matmul(
                        out_ps[:, nh * 512 : (nh + 1) * 512],
                        lhsT=lhsT if lhsT is not None else i128_sb,
                        rhs=prod[:, nh * 512 : (nh + 1) * 512],
                        start=not acc_started[nh],
                        stop=(n_acc_mms[nh] == total_mms),
                    )
                    acc_started[nh] = True

            for p in range(49):
                i, j = p // 7, p % 7
                mode = modes[p]
                w_ps = psW.tile([128, S], F32, tag="w", name="w_ps")
                for nh in range(2):
                    nc.tensor.matmul(
                        w_ps[:, nh * 512 : (nh + 1) * 512],
                        lhsT=c2r_sb[kb : kb + 64, slot, p, :],
                        rhs=t1_sb[kb : kb + 64, slot, nh * 512 : (nh + 1) * 512],
                        start=True,
                        stop=True,
                    )
                wp3 = w_ps.rearrange("q (y x) -> q y x", x=32)
                if mode == "A":
                    prod = pp.tile([128, S], BF16, tag="prod")
                    nc.vector.tensor_mul(
                        out=prod.rearrange("q (y x) -> q y x", x=32),
                        in0=wp3,
                        in1=vpadP4[r][:, i : i + 32, j : j + 32],
                    )
                    pending.append(prod)
                elif mode == "B":
                    w_sb = wsbB.tile([128, S], BF16, tag="wsb")
                    nc.scalar.activation(
                        out=w_sb, in_=w_ps,
                        func=mybir.ActivationFunctionType.Copy,
                    )
                    if j % 2 == 0:
                        vsl = vpadE4[r][:, i : i + 32, j : j + 32]
                    else:
                        vsl = vpadO4[r][:, i : i + 32, j + 1 : j + 33]
                    prod = pp.tile([128, S], BF16, tag="prod")
                    nc.vector.tensor_mul(
                        out=prod.rearrange("q (y x) -> q y x", x=32),
                        in0=w_sb.rearrange("q (y x) -> q y x", x=32),
                        in1=vsl,
                    )
                    pending.append(prod)
                else:  # C: pool path (fp32)
                    w_sb = wsbC.tile([128, S], F32, tag="wsbc")
                    nc.scalar.activation(
                        out=w_sb, in_=w_ps,
                        func=mybir.ActivationFunctionType.Copy,
                    )
                    vsl = vpadP4[r][:, i : i + 32, j : j + 32]
                    if pool_acc is None:
                        pool_acc = accp.tile([128, S], F32R_G, tag="pacc", name="pool_acc")
                        nc.gpsimd.tensor_mul(
                            out=pool_acc.rearrange("q (y x) -> q y x", x=32),
                            in0=w_sb.rearrange("q (y x) -> q y x", x=32),
                            in1=vsl,
                        )
                    else:
                        prodg = pp.tile([128, S], F32, tag="prodg")
                        nc.gpsimd.tensor_mul(
                            out=prodg.rearrange("q (y x) -> q y x", x=32),
                            in0=w_sb.rearrange("q (y x) -> q y x", x=32),
                            in1=vsl,
                        )
                        nc.gpsimd.tensor_add(
                            out=pool_acc, in0=pool_acc, in1=prodg
                        )
                while len(pending) > ACC_LAG:
                    emit_acc(pending.popleft())

            while pending:
                emit_acc(pending.popleft())
            if pool_acc is not None:
                emit_acc(pool_acc, lhsT=i128f_sb)
            # evacuate PSUM, then scatter to DRAM:
            # out channel (g*16+c, r) -> dram row (g*16+c)*4 + r
            out_sb = outs.tile([128, S], F32, tag="out_sb")
            nc.scalar.copy(out=out_sb, in_=out_ps)
            out_view = out_d[:].rearrange("(o r) s -> r o s", r=4)[r]
            nc.sync.dma_start(out=out_view, in_=out_sb)

    nc.compile()
    return nc


_CACHED = {}
LOOP_N = 1  # hwtime.py sets this for looped-NEFF slope timing


def _get_module(loop_n=1, fuse=True):
    key = f"nc{loop_n}_{fuse}"
    if key not in _CACHED:
        _CACHED[key] = _build_module_v2(loop_n)
    return _CACHED[key]


# ------------------------------------------------------------------ entrypoint
def kernel(x, v_w, c1_w, gn_g, gn_b, c2_w, c2_b):
    import ml_dtypes

    x = np.ascontiguousarray(np.asarray(x, np.float32))
    (W1T, WvT, gam_r, bet_r, c2rep, bias_rep, i128, gmat, emat) = _host_prep(
        v_w, c1_w, gn_g, gn_b, c2_w, c2_b
    )

    # v2 drops the per-tap c2 bias (exact only when c2_b == 0, which is how
    # the problem is specified); otherwise fall back to the v1 module
    fuse = bool(np.allclose(np.asarray(c2_b), 0.0))
    if not fuse:
        raise NotImplementedError("nonzero c2 bias not supported by v2 kernel")
    nc = _get_module(loop_n=LOOP_N, fuse=fuse)

    c2rep = c2rep.astype(ml_dtypes.bfloat16)
    i128_bf = i128.astype(ml_dtypes.bfloat16)
    shared = {
        "w1t": W1T,
        "wvt": WvT,
        "c2rep": c2rep,
        "gam": gam_r,
        "bet": bet_r,
        "i128": i128_bf,
        "i128f": np.eye(128, dtype=np.float32),
        "gmat": gmat,
        "emat": emat,
    }
    in_maps = []
    for c in range(NCORES):
        m = dict(shared)
        m["x"] = np.ascontiguousarray(x[c].reshape(512, S))
        in_maps.append(m)

    res = run_bass_kernel_spmd(nc, in_maps, core_ids=list(range(NCORES)))
    _CACHED["last_results"] = res
    out = np.stack([res.results[c]["out"] for c in range(NCORES)])
    return out.reshape(B, 512, H, W)


# revision 13
# speedup vs baseline: 11.7066x; 11.7066x over previous
"""Trainium2 Bass kernel for nn_E4_C4 (C4-equivariant involution CNN).

Contract: kernel(**inputs) takes FULL unsharded inputs (as produced by
setup_inputs) and returns the FULL output [8, 512, 32, 32] fp32.

Strategy (data-parallel over batch, 1 batch element per core, 8 cores):
  per core, channels on partitions, spatial tap-shifts as free-dim offsets
  into zero-padded v planes:
    1. t  = W1 @ x           (PE GEMM fp32r, M=256 K=512 N=1024)
    2. GroupNorm+ReLU        (DVE bn_stats + tiny PE grouping matmuls +
                              ACT per-partition scale/bias apply -> t1 bf16)
    3. v  = Wv @ x           (PE GEMM fp32r) -> three padded 38x38 copies:
                              bf16 even/odd-parity pair (for DVE 2x-mode
                              reads) + fp32 (for Pool/1x paths)
    4. involution, r-PAIR interleaved (r=0/1 share c2 slot 0 on partition
       rows 0-63/64-127 -> consecutive wmap matmuls land on disjoint PE
       row-groups and overlap in the systolic array):
       per tap p and rotation r:
         wrep = c2rep[r,p] @ t1    (PE bf16, K=64, N=1024 -> PSUM fp32;
                                    channel replication + rot90 fused into
                                    host-prepared lhsT)
       then one of (balancing DVE/ACT/Pool):
         A: DVE multiply straight from PSUM (1x) -> prod bf16
         B: ACT evicts PSUM->SBUF bf16, DVE multiplies bf16 pairs (2x mode)
         D: like B but accumulated into an SBUF bf16 accumulator on DVE
         C: ACT evicts fp32, Pool (GPSIMD) multiply+accumulate in SBUF
       A/B prods accumulate into PSUM via PE identity matmuls (bf16 rhs);
       C/D accumulators fold in at the end via identity matmuls.
  Host side: C4-lift of the 1x1 weights, channel reorders, replication,
  rot90 permutations; final gather + channel re-order to reference layout.
"""

import os
from collections import deque
from contextlib import ExitStack

import numpy as np

import concourse.bacc as bacc
import concourse.bass as bass
import concourse.tile as tile
from concourse import mybir
from concourse.bass_utils import run_bass_kernel_spmd

# ---- problem constants (hardcoded per contract) ----
B = 8
CIN = 128
COUT = 128
KK = 7
R = 2
G = 8
GC = 16
H = W = 32
S = H * W  # 1024
EPS = 1e-5
NCORES = 8
F32 = mybir.dt.float32
F32R_G = mybir.dt.float32r
BF16 = mybir.dt.bfloat16

# Per tap-step p the two rotations of the pair are routed through DIFFERENT
# engine chains so the per-step consumers run in parallel:
#   A: DVE multiplies straight from PSUM (1x), B: ACT evicts then DVE 2x,
#   C: ACT evicts fp32 then Pool multiply+accumulate.
# The 7-step rotation balances DVE / ACT / Pool totals.
PATTERN = list(os.environ.get("KRN_PATTERN", "ABABCBA"))
PROD_BUFS = int(os.environ.get("KRN_PROD_BUFS", "8"))
WSB_BUFS = int(os.environ.get("KRN_WSB_BUFS", "3"))
ACC_LAG = int(os.environ.get("KRN_ACC_LAG", "6"))


# ------------------------------------------------------------------ host prep
def _c4_lift_np(w):
    Wr = np.stack([np.roll(w, r, axis=-1) for r in range(4)], axis=1)  # [o,4,i,4]
    o, _, i, _ = Wr.shape
    return Wr.reshape(o * 4, i * 4)


def _host_prep(v_w, c1_w, gn_g, gn_b, c2_w, c2_b):
    W1 = _c4_lift_np(np.asarray(c1_w, np.float32))  # [256, 512], rows c*4+r
    # rows c*4+r -> r-major (r*64+c)
    W1_r = W1.reshape(64, 4, 512).transpose(1, 0, 2).reshape(256, 512)
    W1T = np.ascontiguousarray(W1_r.T)  # [512, 256]

    Wv = _c4_lift_np(np.asarray(v_w, np.float32))  # [512, 512], rows (g*16+c)*4+r
    Wv_r = Wv.reshape(128, 4, 512).transpose(1, 0, 2).reshape(512, 512)
    WvT = np.ascontiguousarray(Wv_r.T)  # [512, 512]

    gam_r = np.ascontiguousarray(
        np.asarray(gn_g, np.float32).reshape(64, 4).T.reshape(2, 128).T
    )  # [128, 2]  col t holds channels t*128..t*128+127 in r-major order
    bet_r = np.ascontiguousarray(
        np.asarray(gn_b, np.float32).reshape(64, 4).T.reshape(2, 128).T
    )

    c2_w = np.asarray(c2_w, np.float32)
    c2_b = np.asarray(c2_b, np.float32)
    c2rep = np.zeros((128, 2, 49, 128), np.float32)
    bias_rep = np.zeros((128, 4, 49), np.float32)
    m_idx = np.arange(128)
    for r in range(4):
        perm = np.rot90(np.arange(49).reshape(7, 7), k=r).flatten()
        base = 64 * (r % 2)
        slot = r // 2
        for p in range(49):
            src_rows = (m_idx // 16) * 49 + perm[p]
            c2rep[base : base + 64, slot, p, :] = c2_w[src_rows, :].T
            bias_rep[:, r, p] = c2_b[src_rows]

    i128 = np.eye(128, dtype=np.float32)
    gmat = np.zeros((128, 64), np.float32)
    gmat[np.arange(128), np.arange(128) % 64] = 0.25
    emat = np.zeros((64, 128), np.float32)
    emat[np.arange(128) % 64, np.arange(128)] = 1.0
    return W1T, WvT, gam_r, bet_r, c2rep, bias_rep, i128, gmat, emat


def _tap_mode(p_idx):
    return PATTERN[p_idx % len(PATTERN)]


# ------------------------------------------------------------------ v2 build
def _build_module_v2(loop_n=1):
    nc = bacc.Bacc(None)

    x_d = nc.dram_tensor("x", [512, S], F32R_G, kind="ExternalInput")
    w1t_d = nc.dram_tensor("w1t", [512, 256], F32R_G, kind="ExternalInput")
    wvt_d = nc.dram_tensor("wvt", [512, 512], F32R_G, kind="ExternalInput")
    c2r_d = nc.dram_tensor("c2rep", [128, 2, 49, 128], BF16, kind="ExternalInput")
    gam_d = nc.dram_tensor("gam", [128, 2], F32, kind="ExternalInput")
    bet_d = nc.dram_tensor("bet", [128, 2], F32, kind="ExternalInput")
    i128_d = nc.dram_tensor("i128", [128, 128], BF16, kind="ExternalInput")
    i128f_d = nc.dram_tensor("i128f", [128, 128], F32R_G, kind="ExternalInput")
    gm_d = nc.dram_tensor("gmat", [128, 64], F32, kind="ExternalInput")
    em_d = nc.dram_tensor("emat", [64, 128], F32, kind="ExternalInput")
    out_d = nc.dram_tensor("out", [512, S], F32, kind="ExternalOutput")

    PAD = 38 * 38  # 1444

    with tile.TileContext(nc) as tc, ExitStack() as ctx:
        if loop_n > 1:
            ctx.enter_context(tc.For_i(0, loop_n, 1))
        consts = ctx.enter_context(tc.tile_pool(name="consts", bufs=1))
        sb = ctx.enter_context(tc.tile_pool(name="sb", bufs=1))
        small = ctx.enter_context(tc.tile_pool(name="small", bufs=8))
        pp = ctx.enter_context(tc.tile_pool(name="pp", bufs=PROD_BUFS))
        wsbB = ctx.enter_context(tc.tile_pool(name="wsbB", bufs=WSB_BUFS))
        wsbC = ctx.enter_context(tc.tile_pool(name="wsbC", bufs=2))
        accp = ctx.enter_context(tc.tile_pool(name="accp", bufs=1))
        outs = ctx.enter_context(tc.tile_pool(name="outs", bufs=2))
        phase1_psum = tc.tile_pool(name="psA", bufs=3, space="PSUM")
        psA = phase1_psum.__enter__()
        stat_psum = tc.tile_pool(name="psStat", bufs=1, space="PSUM")
        psStat = stat_psum.__enter__()

        AL = mybir.AluOpType

        # ---- load weights/constants into SBUF
        x_sb = sb.tile([128, 4, S], F32R_G)
        w1t_sb = sb.tile([128, 4, 256], F32R_G)
        wvt_sb = sb.tile([128, 4, 512], F32R_G)
        dma_engs = [nc.sync, nc.sync, nc.sync, nc.sync]
        for kt in range(4):
            dma_engs[kt].dma_start(
                out=x_sb[:, kt, :], in_=x_d[kt * 128 : (kt + 1) * 128, :]
            )
            dma_engs[(kt + 1) % 4].dma_start(
                out=w1t_sb[:, kt, :], in_=w1t_d[kt * 128 : (kt + 1) * 128, :]
            )
            dma_engs[(kt + 2) % 4].dma_start(
                out=wvt_sb[:, kt, :], in_=wvt_d[kt * 128 : (kt + 1) * 128, :]
            )
        c2r_sb = sb.tile([128, 2, 49, 128], BF16)
        # slot 0 (r=0/1) chunks first so pair-1 taps aren't gated on the rest
        for sl in range(2):
            for pc in range(4):
                ps0, ps1 = pc * 13, min((pc + 1) * 13, 49)
                dma_engs[(sl * 4 + pc) % 4].dma_start(
                    out=c2r_sb[:, sl, ps0:ps1, :], in_=c2r_d[:, sl, ps0:ps1, :]
                )
        gam_sb = consts.tile([128, 2], F32)
        nc.sync.dma_start(out=gam_sb, in_=gam_d[:])
        bet_sb = consts.tile([128, 2], F32)
        nc.sync.dma_start(out=bet_sb, in_=bet_d[:])
        i128_sb = consts.tile([128, 128], BF16)
        nc.sync.dma_start(out=i128_sb, in_=i128_d[:])
        i128f_sb = consts.tile([128, 128], F32R_G)
        nc.sync.dma_start(out=i128f_sb, in_=i128f_d[:])
        gm_sb = consts.tile([128, 64], F32)
        nc.sync.dma_start(out=gm_sb, in_=gm_d[:])
        em_sb = consts.tile([64, 128], F32)
        nc.sync.dma_start(out=em_sb, in_=em_d[:])

        eps_t = consts.tile([64, 1], F32)
        nc.vector.memset(eps_t, EPS)

        # warm the ACT function tables under the DMA shadow
        warm = consts.tile([1, 1], F32)
        nc.vector.memset(warm, 1.0)
        nc.scalar.activation(out=warm, in_=warm, func=mybir.ActivationFunctionType.Relu)
        nc.scalar.activation(out=warm, in_=warm, func=mybir.ActivationFunctionType.Sqrt)

        # padded v planes: bf16 even/odd parity pair + fp32 (pool / 1x paths)
        vpadE = sb.tile([128, 4, PAD], BF16)
        vpadO = sb.tile([128, 4, PAD], BF16)
        vpadP = sb.tile([128, 4, PAD], F32)
        nc.gpsimd.memset(vpadE.bitcast(mybir.dt.uint32), 0)
        nc.gpsimd.memset(vpadO.bitcast(mybir.dt.uint32), 0)
        nc.gpsimd.memset(vpadP, 0.0)

        # ---- GEMM1: t [256, 1024]; both M-tiles stay in PSUM through GN
        ps_t = []
        for mt in range(2):
            pt = psA.tile([128, S], F32, tag="mm_out")
            for nh in range(2):
                for kt in range(4):
                    nc.tensor.matmul(
                        pt[:, nh * 512 : (nh + 1) * 512],
                        lhsT=w1t_sb[:, kt, mt * 128 : (mt + 1) * 128],
                        rhs=x_sb[:, kt, nh * 512 : (nh + 1) * 512],
                        start=(kt == 0),
                        stop=(kt == 3),
                    )
            ps_t.append(pt)

        # ---- GroupNorm stats (read PSUM directly; m2 assembled in one STT)
        stats = []
        for t in range(2):
            st6 = small.tile([128, 2, 6], F32, tag="st6")
            for hh in range(2):
                nc.vector.bn_stats(
                    out=st6[:, hh, :], in_=ps_t[t][:, hh * 512 : (hh + 1) * 512]
                )
            mv = small.tile([128, 2], F32, tag="mv")
            nc.vector.bn_aggr(out=mv, in_=st6)
            # mv[:,1] <- mean^2 + var  (in-place; mv becomes [mean, m2])
            nc.vector.scalar_tensor_tensor(
                out=mv[:, 1:2],
                in0=mv[:, 0:1],
                scalar=mv[:, 0:1],
                in1=mv[:, 1:2],
                op0=AL.mult,
                op1=AL.add,
            )
            stats.append(mv)

        def emit_gemmv(r):
            ps_v = psA.tile([128, S], F32, tag="mm_out", name=f"ps_v{r}")
            for nh in range(2):
                for kt in range(4):
                    nc.tensor.matmul(
                        ps_v[:, nh * 512 : (nh + 1) * 512],
                        lhsT=wvt_sb[:, kt, r * 128 : (r + 1) * 128],
                        rhs=x_sb[:, kt, nh * 512 : (nh + 1) * 512],
                        start=(kt == 0),
                        stop=(kt == 3),
                    )
            ps_v3 = ps_v.rearrange("q (y x) -> q y x", x=32)
            for vt, c0 in ((vpadE, 3), (vpadO, 4), (vpadP, 3)):
                vint = vt[:, r, :].rearrange("q (yy xx) -> q yy xx", xx=38)[
                    :, 3:35, c0 : c0 + 32
                ]
                nc.scalar.activation(
                    out=vint, in_=ps_v3, func=mybir.ActivationFunctionType.Copy
                )

        # GEMMv r=0 runs in the third psA buffer while the GN scalar chain
        # (DVE/ACT) works through the group stats, keeping the PE busy
        emit_gemmv(0)

        ps_g = psStat.tile([64, 2], F32, tag="gstat")
        for t in range(2):
            nc.tensor.matmul(
                ps_g, lhsT=gm_sb, rhs=stats[t], start=(t == 0), stop=(t == 1)
            )
        # group mean / m2 -> rstd
        gss = small.tile([64, 2], F32, tag="gss")
        nc.vector.tensor_copy(out=gss, in_=ps_g)  # evacuate PSUM
        gmv = small.tile([64, 2], F32, tag="gmv")  # [mean_g, rstd_g]
        nc.vector.tensor_copy(out=gmv[:, 0:1], in_=gss[:, 0:1])
        gv = small.tile([64, 1], F32, tag="gv")
        nc.vector.tensor_mul(out=gv, in0=gss[:, 0:1], in1=gss[:, 0:1])
        nc.vector.tensor_sub(out=gv, in0=gss[:, 1:2], in1=gv)
        nc.scalar.activation(
            out=gv, in_=gv, func=mybir.ActivationFunctionType.Sqrt, bias=eps_t, scale=1.0
        )
        nc.vector.reciprocal(out=gmv[:, 1:2], in_=gv)

        ps_e = psStat.tile([128, 2], F32, tag="gstat")
        nc.tensor.matmul(ps_e, lhsT=em_sb, rhs=gmv, start=True, stop=True)

        # per-partition scale/bias; apply GN + ReLU into t1
        t1_sb = sb.tile([128, 2, S], BF16)
        scb = small.tile([128, 2, 2], F32, tag="scb")
        for t in range(2):
            nc.vector.tensor_mul(
                out=scb[:, t, 0:1], in0=ps_e[:, 1:2], in1=gam_sb[:, t : t + 1]
            )
            nc.vector.tensor_mul(out=scb[:, t, 1:2], in0=ps_e[:, 0:1], in1=scb[:, t, 0:1])
            nc.vector.tensor_sub(
                out=scb[:, t, 1:2], in0=bet_sb[:, t : t + 1], in1=scb[:, t, 1:2]
            )
            nc.scalar.activation(
                out=t1_sb[:, t, :],
                in_=ps_t[t][:, :],
                func=mybir.ActivationFunctionType.Relu,
                scale=scb[:, t, 0:1],
                bias=scb[:, t, 1:2],
            )

        # remaining GEMMv tiles reuse the ps_t buffers as GN releases them
        for r in range(1, 4):
            emit_gemmv(r)

        # phase-1 PSUM pools close here; the involution reuses their banks
        stat_psum.__exit__(None, None, None)
        phase1_psum.__exit__(None, None, None)
        psW = ctx.enter_context(tc.tile_pool(name="psW", bufs=3, space="PSUM"))
        psO = ctx.enter_context(tc.tile_pool(name="psO", bufs=1, space="PSUM"))

        vpadE4 = [vpadE[:, r, :].rearrange("q (yy xx) -> q yy xx", xx=38) for r in range(4)]
        vpadO4 = [vpadO[:, r, :].rearrange("q (yy xx) -> q yy xx", xx=38) for r in range(4)]
        vpadP4 = [vpadP[:, r, :].rearrange("q (yy xx) -> q yy xx", xx=38) for r in range(4)]

        # ---- involution: one rotation r at a time; psW ring-3 gives the
        # PE three taps of lookahead so the per-tap chain latency
        # (wmap matmul -> ACT/DVE consume -> PE accumulate) stays hidden
        for r in range(4):
            slot = r // 2
            kb = 64 * (r % 2)
            out_ps = psO.tile([128, S], F32, tag="o", name="out_ps")
            acc_started = [False, False]
            n_acc_mms = [0, 0]
            pool_acc = None
            modes = [_tap_mode(p) for p in range(49)]
            # accumulation groups are per PSUM bank: each spatial half of
            # out_ps needs its own start=True first matmul and stop=True last
            total_mms = sum(m in ("A", "B") for m in modes) + ("C" in modes)

            pending = deque()  # prod tiles awaiting PE accumulation

            def emit_acc(prod, lhsT=None):
                for nh in range(2):
                    n_acc_mms[nh] += 1
                    nc.tensor.matmul(
                        out_ps[:, nh * 512 : (nh + 1) * 512],
                        lhsT=lhsT if lhsT is not None else i128_sb,
                        rhs=prod[:, nh * 512 : (nh + 1) * 512],
                        start=not acc_started[nh],
                        stop=(n_acc_mms[nh] == total_mms),
                    )
                    acc_started[nh] = True

            for p in range(49):
                i, j = p // 7, p % 7
                mode = modes[p]
                w_ps = psW.tile([128, S], F32, tag="w", name="w_ps")
                for nh in range(2):
                    nc.tensor.matmul(
                        w_ps[:, nh * 512 : (nh + 1) * 512],
                        lhsT=c2r_sb[kb : kb + 64, slot, p, :],
                        rhs=t1_sb[kb : kb + 64, slot, nh * 512 : (nh + 1) * 512],
                        start=True,
                        stop=True,
                    )
                wp3 = w_ps.rearrange("q (y x) -> q y x", x=32)
                if mode == "A":
                    prod = pp.tile([128, S], BF16, tag="prod")
                    nc.vector.tensor_mul(
                        out=prod.rearrange("q (y x) -> q y x", x=32),
                        in0=wp3,
                        in1=vpadP4[r][:, i : i + 32, j : j + 32],
                    )
                    pending.append(prod)
                elif mode == "B":
                    w_sb = wsbB.tile([128, S], BF16, tag="wsb")
                    nc.scalar.activation(
                        out=w_sb, in_=w_ps,
                        func=mybir.ActivationFunctionType.Copy,
                    )
                    if j % 2 == 0:
                        vsl = vpadE4[r][:, i : i + 32, j : j + 32]
                    else:
                        vsl = vpadO4[r][:, i : i + 32, j + 1 : j + 33]
                    prod = pp.tile([128, S], BF16, tag="prod")
                    nc.vector.tensor_mul(
                        out=prod.rearrange("q (y x) -> q y x", x=32),
                        in0=w_sb.rearrange("q (y x) -> q y x", x=32),
                        in1=vsl,
                    )
                    pending.append(prod)
                else:  # C: pool path (fp32)
                    w_sb = wsbC.tile([128, S], F32, tag="wsbc")
                    nc.scalar.activation(
                        out=w_sb, in_=w_ps,
                        func=mybir.ActivationFunctionType.Copy,
                    )
                    vsl = vpadP4[r][:, i : i + 32, j : j + 32]
                    if pool_acc is None:
                        pool_acc = accp.tile([128, S], F32R_G, tag="pacc", name="pool_acc")
                        nc.gpsimd.tensor_mul(
                            out=pool_acc.rearrange("q (y x) -> q y x", x=32),
                            in0=w_sb.rearrange("q (y x) -> q y x", x=32),
                            in1=vsl,
                        )
                    else:
                        prodg = pp.tile([128, S], F32, tag="prodg")
                        nc.gpsimd.tensor_mul(
                            out=prodg.rearrange("q (y x) -> q y x", x=32),
                            in0=w_sb.rearrange("q (y x) -> q y x", x=32),
                            in1=vsl,
                        )
                        nc.gpsimd.tensor_add(
                            out=pool_acc, in0=pool_acc, in1=prodg
                        )
                while len(pending) > ACC_LAG:
                    emit_acc(pending.popleft())

            while pending:
                emit_acc(pending.popleft())
            if pool_acc is not None:
                emit_acc(pool_acc, lhsT=i128f_sb)
            # evacuate PSUM, then scatter to DRAM:
            # out channel (g*16+c, r) -> dram row (g*16+c)*4 + r
            out_sb = outs.tile([128, S], F32, tag="out_sb")
            nc.scalar.copy(out=out_sb, in_=out_ps)
            out_view = out_d[:].rearrange("(o r) s -> r o s", r=4)[r]
            nc.sync.dma_start(out=out_view, in_=out_sb)

    nc.compile()
    return nc


_CACHED = {}
LOOP_N = 1  # hwtime.py sets this for looped-NEFF slope timing


def _get_module(loop_n=1, fuse=True):
    key = f"nc{loop_n}_{fuse}"
    if key not in _CACHED:
        _CACHED[key] = _build_module_v2(loop_n)
    return _CACHED[key]


# ------------------------------------------------------------------ entrypoint
def kernel(x, v_w, c1_w, gn_g, gn_b, c2_w, c2_b):
    import ml_dtypes

    x = np.ascontiguousarray(np.asarray(x, np.float32))
    (W1T, WvT, gam_r, bet_r, c2rep, bias_rep, i128, gmat, emat) = _host_prep(
        v_w, c1_w, gn_g, gn_b, c2_w, c2_b
    )

    # v2 drops the per-tap c2 bias (exact only when c2_b == 0, which is how
    # the problem is specified); otherwise fall back to the v1 module
    fuse = bool(np.allclose(np.asarray(c2_b), 0.0))
    if not fuse:
        raise NotImplementedError("nonzero c2 bias not supported by v2 kernel")
    nc = _get_module(loop_n=LOOP_N, fuse=fuse)

    c2rep = c2rep.astype(ml_dtypes.bfloat16)
    i128_bf = i128.astype(ml_dtypes.bfloat16)
    shared = {
        "w1t": W1T,
        "wvt": WvT,
        "c2rep": c2rep,
        "gam": gam_r,
        "bet": bet_r,
        "i128": i128_bf,
        "i128f": np.eye(128, dtype=np.float32),
        "gmat": gmat,
        "emat": emat,
    }
    in_maps = []
    for c in range(NCORES):
        m = dict(shared)
        m["x"] = np.ascontiguousarray(x[c].reshape(512, S))
        in_maps.append(m)

    res = run_bass_kernel_spmd(nc, in_maps, core_ids=list(range(NCORES)))
    _CACHED["last_results"] = res
    out = np.stack([res.results[c]["out"] for c in range(NCORES)])
    return out.reshape(B, 512, H, W)
